# revision 18
# baseline (speedup 1.0000x reference)
"""TGCN (dense-graph GRU) Trainium2 kernel, 8-core SPMD, no collectives.

Math (per reference):
  xh_t = relu(x_t @ fc_w + fc_b)                    [N, H]
  S_t  = adj @ xh_t                                 (assoc: adj@(xh@W) = (adj@xh)@W)
  z_t  = sigmoid(S_t @ Mz + h @ Lz_bot + bz)        Mz = Wz @ Lz_top (host-folded)
  r_t  = sigmoid(S_t @ Mr + h @ Lr_bot + br)
  ht_t = tanh   (S_t @ Mh + (h*r) @ Lh_bot + bh)
  h'   = z*h + (1-z)*ht = (h - ZC*h) + ZC*ht        ZC := 1-z

Sharding: row-partition adj across 8 cores (512 nodes each). The GRU cell is
row-local, so each core runs the whole time loop on its shard independently.
x is replicated (an all-gather per step would be latency-bound, ~10us/64KB).

Layout: feature-major on-chip: S.T, h.T are [64 feat, 512 nodes]; steps
processed in pairs so the PE operands use the full 128-partition dim.

Perf structure (final, 280us HW vs 362us baseline):
 - xh matmuls stacked-K: stationary [xt_s0; xt_s1] (K=128 = 2 steps x 64
   feat), moving block-diag [fcw 0; 0 fcw] -> one N=128 matmul per 128-node
   tile covers BOTH steps. Measured 56ns/MM sustained (was 64 LDW-bound
   N=64 MMs at ~2x the cost).
 - S matmul fp8 e4m3 perf_mode=DoubleRow (2 K-tiles per MM): 16 MMs/pair.
   adj x4096, xh x16 to sit in e4m3 range; 1/65536 folded into Mz/Mr/Mh.
   Measured 216-270ns/MM warm (1.9x over bf16). The 4-MM bursts are
   spread through the pair (after each group's relu) to keep PE activity
   dense -- long PE-idle stretches re-throttle the HAM clock to 1.2GHz.
 - Gates: z-columns of the zr stationary are NEGATED so ONE activation
   computes [ZC | R] = sigmoid(ps + [-bz; br]). The h-candidate matmul is
   split K=64+K=64 (Mh@S early + Lh@(h*r) late, accumulating), so no
   [S; h*r] concat tile or its PSUM->SBUF copy exists.
 - Chain placement is everything: the per-step GRU chain
   (zr-MM -> sigmoid -> h*r -> lh-MM -> tanh -> ZC*ht -> add) is the
   critical path (~5us/step). All chain TTs live on DVE (~420ns each);
   GPSIMD is unused for compute (its TT is 1.16us AND it contends for
   the DVE SBUF port). Relus run on ACT, each emitted so it lands in a
   chain gap of the strict-FIFO ACT queue, never ahead of an imminent
   sigmoid/tanh; chain matmuls are emitted ahead of S-bursts so they
   don't queue behind ~1us of bulk PE work. A TT with operands at
   different partition bases fails walrus codegen -- hence the one ZC
   realign copy (ACT cross-base activations are fine).
 - Combine h' = (h - ZC*h) + ZC*ht: ZC*h and the subtract run off-chain.
 - xt DMA issuance alternates scalar/sync queues: the dma_start's WAR
   wait (xt pool bufs=2 -> waits ~2 pairs) blocks the issuing engine's
   FIFO, so consecutive pairs must use different queues (all-on-sync
   serializes the waits: 351us; gpsimd issuance also regressed).

Measured HW exec: 280-281us (vs 362us baseline), rel err 7.0e-3.
"""

import os
import sys

sys.path.insert(0, "/opt/trn_rl_repo")

import numpy as np
import ml_dtypes

T, N, F_IN, H1, F_OUT = 48, 4096, 64, 64, 64
NCORES = 8
NS = N // NCORES          # nodes per core = 512
PAIRS = T // 2            # 24
KT = N // 128             # 32 node k-tiles
ADJ_SCALE = 4096.0        # adj -> e4m3 range
XH_SCALE = 16.0           # xh -> e4m3 range
FCW_SCALE = 32.0          # fcw -> e4m3 range

_cache = {}


def _build():
    import concourse.bass as bass
    import concourse.mybir as mybir
    import concourse.tile as tile
    from concourse import bacc

    f32 = mybir.dt.float32
    bf16 = mybir.dt.bfloat16
    fp8 = mybir.dt.float8e4
    AF = mybir.ActivationFunctionType
    ALU = mybir.AluOpType
    DR = mybir.MatmulPerfMode.DoubleRow

    nc = bacc.Bacc(
        "TRN2",
        target_bir_lowering=False,
        debug=False,
        enable_asserts=False,
        num_devices=NCORES,
    )

    # DRAM parameters (per-core shapes)
    adjT_d = nc.dram_tensor("adjT", [128, KT, NS], fp8, kind="ExternalInput").ap()
    xT_d = nc.dram_tensor("xT", [PAIRS, 128, N], fp8, kind="ExternalInput").ap()
    fcw_d = nc.dram_tensor("fcw", [128, 128], fp8, kind="ExternalInput").ap()
    wzr_d = nc.dram_tensor("wzr", [128, 128], bf16, kind="ExternalInput").ap()
    mh_d = nc.dram_tensor("mh", [F_OUT, F_OUT], bf16, kind="ExternalInput").ap()
    lh_d = nc.dram_tensor("lh", [F_OUT, F_OUT], bf16, kind="ExternalInput").ap()
    bzr_d = nc.dram_tensor("bzr", [128, 1], f32, kind="ExternalInput").ap()
    bh_d = nc.dram_tensor("bh", [F_OUT, 1], f32, kind="ExternalInput").ap()
    out_d = nc.dram_tensor("out", [F_OUT, NS], f32, kind="ExternalOutput").ap()

    with tile.TileContext(nc) as tc:
        with (
            tc.tile_pool(name="const", bufs=1) as constp,
            tc.tile_pool(name="state", bufs=1) as statep,
            tc.tile_pool(name="xt", bufs=2) as xtp,
            tc.tile_pool(name="xh", bufs=2) as xhp,
            tc.tile_pool(name="gw", bufs=3) as gwp,
            tc.tile_pool(name="psx", bufs=2, space="PSUM") as psxp,
            tc.tile_pool(name="pss", bufs=2, space="PSUM") as pssp,
            tc.tile_pool(name="pszr", bufs=1, space="PSUM") as pszrp,
            tc.tile_pool(name="psh", bufs=1, space="PSUM") as pshp,
        ):
            # ---- constants ----
            fcw_sb = constp.tile([128, 128], fp8)
            nc.sync.dma_start(out=fcw_sb[:], in_=fcw_d[:])
            adjT_sb = constp.tile([128, KT, NS], fp8)
            for q, eng in enumerate((nc.sync, nc.gpsimd, nc.gpsimd, nc.gpsimd)):
                eng.dma_start(
                    out=adjT_sb[:, q * 8 : (q + 1) * 8, :],
                    in_=adjT_d[:, q * 8 : (q + 1) * 8, :],
                )
            wzr_sb = constp.tile([128, 128], bf16)
            mh_sb = constp.tile([F_OUT, F_OUT], bf16)
            lh_sb = constp.tile([F_OUT, F_OUT], bf16)
            bzr_sb = constp.tile([128, 1], f32)
            bh_sb = constp.tile([F_OUT, 1], f32)
            for dst, src in (
                (wzr_sb, wzr_d), (mh_sb, mh_d), (lh_sb, lh_d),
                (bzr_sb, bzr_d), (bh_sb, bh_d),
            ):
                nc.gpsimd.dma_start(out=dst[:], in_=src[:])

            # ---- state ----
            # Concat rhs tiles for the K=128 zr matmul: rows 0-63 carry S_t.T
            # (refreshed per pair, off-chain), rows 64-127 h.T; the combine
            # writes h' straight into the next buffer's bottom (4-rotation).
            CzS = []
            for i in range(4):
                czsi = statep.tile([128, NS], bf16, tag=f"CzS{i}")
                CzS.append(czsi)
            nc.vector.memset(CzS[0][:], 0.0)

            def emit_xh_group(xt, xh, g, eng):
                # one stacked-K matmul per 128-node tile: out[128 nodes,
                # 128 = s0 feats | s1 feats]; 8 node tiles fill 2 PSUM banks
                ps = psxp.tile([128, 1024], mybir.dt.float32)
                for j in range(8):
                    k = 8 * g + j
                    nc.tensor.matmul(
                        ps[:, j * 128 : (j + 1) * 128],
                        lhsT=xt[:, k * 128 : (k + 1) * 128],
                        rhs=fcw_sb[:],
                        start=True, stop=True,
                    )
                # xh16 = max((XH/FCW)*ps, 0) -> fp8
                dst = xh[:, 8 * g : 8 * (g + 1), :].rearrange("p a b -> p (a b)")
                if eng == "act":
                    nc.scalar.activation(dst, ps[:], AF.Relu,
                                         scale=XH_SCALE / FCW_SCALE)
                else:
                    nc.vector.tensor_scalar(dst, ps[:], XH_SCALE / FCW_SCALE,
                                            0.0, ALU.mult, ALU.max)

            def emit_gru_front(step):
                # zr matmul + combined [ZC | R] sigmoid + h*r + early Mh@S
                cur = CzS[step % 4]
                H = cur[64:128, :]

                ps_h = pshp.tile([F_OUT, NS], mybir.dt.float32, tag="ps_h")
                # early half of the h-candidate: Mh @ S (off-chain: S has
                # been sitting in cur[0:64] since the previous pair)
                nc.tensor.matmul(ps_h[:], lhsT=mh_sb[:], rhs=cur[0:64, :],
                                 start=True, stop=False)
                ps_zr = pszrp.tile([128, NS], mybir.dt.float32, tag="ps_zr")
                nc.tensor.matmul(ps_zr[:], lhsT=wzr_sb[:], rhs=cur[:],
                                 start=True, stop=True)
                # G = sigmoid(ps_zr + [-bz; br]): ZC = 1-z in rows 0-63
                # (z-columns of wzr are negated host-side), R in rows 64-127
                G = gwp.tile([128, NS], bf16, tag="G")
                nc.scalar.activation(G[:], ps_zr[:], AF.Sigmoid, bias=bzr_sb[:])
                # h*r -> base-0 tile (rhs of the late h-candidate matmul)
                ch = gwp.tile([F_OUT, NS], bf16, tag="ch")
                nc.vector.tensor_mul(ch[:], H, G[64:128, :])
                # realign ZC to partitions 64-127 for the combine (off-chain;
                # a TT with operands at different partition bases fails walrus
                # codegen, so the copy is unavoidable)
                ZC = gwp.tile([128, NS], bf16, tag="ZC")
                nc.vector.tensor_copy(ZC[64:128, :], G[0:64, :])
                G2 = gwp.tile([128, NS], bf16, tag="G2")
                nc.vector.tensor_mul(G2[64:128, :], ZC[64:128, :], H)
                P = gwp.tile([128, NS], bf16, tag="P")
                nc.vector.tensor_sub(P[64:128, :], H, G2[64:128, :])
                return ps_h, ch, ZC, P

            def emit_gru_back(step, ps_h, ch, ZC, P):
                # late h-candidate matmul + tanh + on-chain combine tail.
                # high_priority pins the chain matmul ahead of the S-bursts
                # in the scheduler's PE ordering -- otherwise it lands after
                # ~2us of bulk work and the tanh (and whole second step)
                # stalls behind it
                nxt = CzS[(step + 1) % 4]
                with tc.high_priority(offset=32):
                    nc.tensor.matmul(ps_h[:], lhsT=lh_sb[:], rhs=ch[:],
                                     start=False, stop=True)
                HT = gwp.tile([128, NS], bf16, tag="HT")
                nc.scalar.activation(HT[64:128, :], ps_h[:], AF.Tanh,
                                     bias=bh_sb[:])
                G1 = gwp.tile([128, NS], bf16, tag="G1")
                nc.vector.tensor_mul(G1[64:128, :], ZC[64:128, :], HT[64:128, :])
                nc.vector.tensor_add(nxt[64:128, :], P[64:128, :], G1[64:128, :])

            # ---- main loop, software-pipelined: gates of pair p-1 are
            # emitted between the xh matmul groups of pair p ----
            for p in range(PAIRS):
                xt = xtp.tile([128, N], fp8)
                (nc.scalar if p % 2 == 0 else nc.sync).dma_start(
                    out=xt[:], in_=xT_d[p]
                )
                xh = xhp.tile([128, KT, 128], fp8)

                # S-pair matmul accumulator: psS[2*64 feat, 512 my-nodes],
                # 16 fp8 DoubleRow k-tile-pair MMs. Each DR MM k only needs
                # xh group k//4, so the four 4-MM bursts are interleaved
                # right after their group's relu — PE work stays dense
                # through the whole pair (no HAM re-throttle) instead of
                # bunching at the end.
                psS = pssp.tile([128, NS], mybir.dt.float32)

                def emit_s_burst(g):
                    for k in range(4 * g, 4 * g + 4):
                        nc.tensor.matmul(
                            psS[:],
                            lhsT=xh[:, 2 * k : 2 * k + 2, :],
                            rhs=adjT_sb[:, 2 * k : 2 * k + 2, :],
                            start=(k == 0), stop=(k == KT // 2 - 1),
                            perf_mode=DR,
                        )

                # order: each group's relu lands in the ACT queue BEFORE the
                # next chain op (sig/tanh), so bulk work fills chain gaps
                # instead of head-of-line blocking behind a waiting chain op
                # PE-queue placement: the chain matmuls (zr, lh) go BEFORE
                # each S-burst so they aren't stuck behind ~1us of bulk PE
                # work in the in-order FIFO when their chain deps resolve;
                # each group's relu still precedes the next chain sig/tanh
                # in the ACT queue (fills the chain gaps without blocking).
                emit_xh_group(xt, xh, 0, "act")
                if p >= 1:
                    fr0 = emit_gru_front(2 * p - 2)
                emit_xh_group(xt, xh, 1, "act")
                if p >= 1:
                    emit_gru_back(2 * p - 2, *fr0)
                emit_s_burst(0)
                if p >= 1:
                    fr1 = emit_gru_front(2 * p - 1)
                emit_xh_group(xt, xh, 2, "act")
                emit_s_burst(1)
                if p >= 1:
                    emit_gru_back(2 * p - 1, *fr1)
                emit_xh_group(xt, xh, 3, "act")
                emit_s_burst(2)
                emit_s_burst(3)
                # refresh concat tops for this pair's two steps (buffers
                # last read two pairs ago -> fully off the gate chain)
                s0, s1 = (2 * p) % 4, (2 * p + 1) % 4
                nc.vector.tensor_copy(CzS[s0][0:64, :], psS[0:64, :])
                nc.vector.tensor_copy(CzS[s1][0:64, :], psS[64:128, :])

            # drain: gates for the last pair
            fr = emit_gru_front(2 * PAIRS - 2)
            emit_gru_back(2 * PAIRS - 2, *fr)
            fr = emit_gru_front(2 * PAIRS - 1)
            emit_gru_back(2 * PAIRS - 1, *fr)

            Hout = statep.tile([F_OUT, NS], f32)
            nc.scalar.copy(Hout[:], CzS[(2 * PAIRS) % 4][64:128, :])
            nc.sync.dma_start(out=out_d[:], in_=Hout[:])

    nc.compile()
    return nc


def _prep_inputs(x, adj, fc_w, Wz, Wr, Wh, Lz, Lr, Lh, bz, br, bh):
    bf16 = ml_dtypes.bfloat16
    fp8 = ml_dtypes.float8_e4m3
    f32 = np.float32

    # x [T, N, F] -> [PAIRS, 2*F, N]: partition dim = (step, feat)
    xT = np.ascontiguousarray(
        x.reshape(PAIRS, 2, N, F_IN).transpose(0, 1, 3, 2).reshape(PAIRS, 128, N)
    ).astype(fp8)
    # block-diag [fcw 0; 0 fcw] so one matmul covers both steps
    fcw_stack = np.zeros((128, 128), f32)
    fcw_stack[0:64, 0:64] = fc_w * FCW_SCALE
    fcw_stack[64:128, 64:128] = fc_w * FCW_SCALE
    fcw_stack = fcw_stack.astype(fp8)

    gate_scale = 1.0 / (ADJ_SCALE * XH_SCALE)  # S arrives x65536

    def fold(W, L):
        m = W.astype(np.float64) @ L[:F_OUT].astype(np.float64)
        return (m * gate_scale).astype(np.float64)

    mz, mr, mh = fold(Wz, Lz), fold(Wr, Lr), fold(Wh, Lh)
    # zr stationary: z-columns NEGATED so sigmoid gives ZC = 1-z directly
    wzr = np.zeros((128, 128), np.float64)
    wzr[0:64, 0:64] = -mz
    wzr[64:128, 0:64] = -Lz[F_OUT:].astype(np.float64)
    wzr[0:64, 64:128] = mr
    wzr[64:128, 64:128] = Lr[F_OUT:].astype(np.float64)
    bzr = np.concatenate([-bz, br]).reshape(128, 1)
    shared = {
        "xT": xT, "fcw": fcw_stack,
        "wzr": wzr.astype(bf16),
        "mh": mh.astype(bf16),
        "lh": Lh[F_OUT:].astype(bf16),
        "bzr": bzr.astype(f32),
        "bh": bh.reshape(F_OUT, 1).astype(f32),
    }
    in_maps = []
    for c in range(NCORES):
        m = dict(shared)
        at = adj[c * NS : (c + 1) * NS, :].T * ADJ_SCALE  # [N, NS]
        m["adjT"] = np.ascontiguousarray(
            at.reshape(KT, 128, NS).transpose(1, 0, 2)
        ).astype(fp8)
        in_maps.append(m)
    return in_maps


def kernel(x, adj, fc_w, fc_b, Wz, Wr, Wh, Lz, Lr, Lh, bz, br, bh):
    x = np.asarray(x, np.float32)
    adj = np.asarray(adj, np.float32)
    args = [np.asarray(a, np.float32) for a in (fc_w, Wz, Wr, Wh, Lz, Lr, Lh, bz, br, bh)]
    fc_b = np.asarray(fc_b, np.float32)
    if np.any(fc_b != 0.0):
        # fc_b can't fold into the per-partition activation bias (it varies
        # along the free dim); the reference always passes zeros. Pure-numpy
        # fallback keeps kernel() correct for arbitrary inputs.
        return _numpy_ref(x, adj, args[0], fc_b, *args[1:])

    from concourse.bass_utils import run_bass_kernel_spmd

    if "nc" not in _cache:
        _cache["nc"] = _build()
    nc = _cache["nc"]

    in_maps = _prep_inputs(x, adj, *args)
    trace = bool(int(os.environ.get("BASS_KERNEL_TRACE", "0")))
    kwargs = {}
    if trace:
        _install_trace_shim()
        tmpdir = os.environ.get("BASS_KERNEL_TRACE_DIR")
        if tmpdir:
            os.makedirs(tmpdir, exist_ok=True)
            kwargs["tmpdir"] = tmpdir
    res = run_bass_kernel_spmd(
        nc, in_maps, core_ids=list(range(NCORES)), trace=trace, **kwargs
    )
    _cache["last_result"] = res

    out = np.empty((1, N, F_OUT), np.float32)
    for c in range(NCORES):
        out[0, c * NS : (c + 1) * NS, :] = res.results[c]["out"].T
    return out


def _install_trace_shim():
    """Register the NTFF profile hook (this image's antenv lacks axon_hooks)
    and stub out the artifact upload so profiling works offline."""
    import types

    try:
        from antenv import axon_hooks  # noqa: F401
        return
    except ImportError:
        pass
    sys.path.insert(0, "/root/.axon_site")
    from trn_agent_boot.trn_boot import _ntff_profile_via_ctypes

    hook = _ntff_profile_via_ctypes("/opt/axon/libaxon_pjrt.so")
    m = types.ModuleType("antenv.axon_hooks")
    m.get_axon_ntff_profile_hook = lambda: hook
    m.set_axon_ntff_profile_hook = lambda h: None
    sys.modules["antenv.axon_hooks"] = m
    import antenv

    antenv.axon_hooks = m
    from concourse import bass_utils as _bu

    _bu.upload_artifacts = lambda tmpdir: tmpdir


def _numpy_ref(x, adj, fc_w, fc_b, Wz, Wr, Wh, Lz, Lr, Lh, bz, br, bh):
    def sigmoid(v):
        return 1.0 / (1.0 + np.exp(-v))

    xh = np.maximum(x @ fc_w + fc_b, 0.0)
    h = np.zeros((N, F_OUT), np.float32)
    for t in range(T):
        s = adj @ xh[t]
        az, ar, ah = s @ Wz, s @ Wr, s @ Wh
        z = sigmoid(np.concatenate([az, h], -1) @ Lz + bz)
        r = sigmoid(np.concatenate([ar, h], -1) @ Lr + br)
        ht = np.tanh(np.concatenate([ah, h * r], -1) @ Lh + bh)
        h = z * h + (1.0 - z) * ht
    return h[None].astype(np.float32)


# revision 20
# speedup vs baseline: 1.1628x; 1.1628x over previous
"""TGCN (dense-graph GRU) Trainium2 kernel, 8-core SPMD, no collectives.

Math (per reference):
  xh_t = relu(x_t @ fc_w + fc_b)                    [N, H]
  S_t  = adj @ xh_t                                 (assoc: adj@(xh@W) = (adj@xh)@W)
  z_t  = sigmoid(S_t @ Mz + h @ Lz_bot + bz)        Mz = Wz @ Lz_top (host-folded)
  r_t  = sigmoid(S_t @ Mr + h @ Lr_bot + br)
  ht_t = tanh   (S_t @ Mh + (h*r) @ Lh_bot + bh)
  h'   = z*h + (1-z)*ht = (h - ZC*h) + ZC*ht        ZC := 1-z

Sharding: row-partition adj across 8 cores (512 nodes each). The GRU cell is
row-local, so each core runs the whole time loop on its shard independently.
x is replicated (an all-gather per step would be latency-bound, ~10us/64KB).

Layout: feature-major on-chip: S.T, h.T are [64 feat, 512 nodes]; steps
processed in pairs so the PE operands use the full 128-partition dim.

Perf structure (final, 280us HW vs 362us baseline):
 - xh matmuls stacked-K: stationary [xt_s0; xt_s1] (K=128 = 2 steps x 64
   feat), moving block-diag [fcw 0; 0 fcw] -> one N=128 matmul per 128-node
   tile covers BOTH steps. Measured 56ns/MM sustained (was 64 LDW-bound
   N=64 MMs at ~2x the cost).
 - S matmul fp8 e4m3 perf_mode=DoubleRow (2 K-tiles per MM): 16 MMs/pair.
   adj x4096, xh x16 to sit in e4m3 range; 1/65536 folded into Mz/Mr/Mh.
   Measured 216-270ns/MM warm (1.9x over bf16). The 4-MM bursts are
   spread through the pair (after each group's relu) to keep PE activity
   dense -- long PE-idle stretches re-throttle the HAM clock to 1.2GHz.
 - Gates: z-columns of the zr stationary are NEGATED so ONE activation
   computes [ZC | R] = sigmoid(ps + [-bz; br]). The h-candidate matmul is
   split K=64+K=64 (Mh@S early + Lh@(h*r) late, accumulating), so no
   [S; h*r] concat tile or its PSUM->SBUF copy exists.
 - Chain placement is everything: the per-step GRU chain
   (zr-MM -> sigmoid -> h*r -> lh-MM -> tanh -> ZC*ht -> add) is the
   critical path (~5us/step). All chain TTs live on DVE (~420ns each);
   GPSIMD is unused for compute (its TT is 1.16us AND it contends for
   the DVE SBUF port). Relus run on ACT, each emitted so it lands in a
   chain gap of the strict-FIFO ACT queue, never ahead of an imminent
   sigmoid/tanh; chain matmuls are emitted ahead of S-bursts so they
   don't queue behind ~1us of bulk PE work. A TT with operands at
   different partition bases fails walrus codegen -- hence the one ZC
   realign copy (ACT cross-base activations are fine).
 - Combine h' = (h - ZC*h) + ZC*ht: ZC*h and the subtract run off-chain.
 - xt DMA issuance alternates scalar/sync queues: the dma_start's WAR
   wait (xt pool bufs=2 -> waits ~2 pairs) blocks the issuing engine's
   FIFO, so consecutive pairs must use different queues (all-on-sync
   serializes the waits: 351us; gpsimd issuance also regressed).

Measured HW exec: 280-281us (vs 362us baseline), rel err 7.0e-3.
"""

import os
import sys

sys.path.insert(0, "/opt/trn_rl_repo")

import numpy as np
import ml_dtypes

T, N, F_IN, H1, F_OUT = 48, 4096, 64, 64, 64
NCORES = 8
NS = N // NCORES          # nodes per core = 512
PAIRS = T // 2            # 24
KT = N // 128             # 32 node k-tiles
ADJ_SCALE = 4096.0        # adj -> e4m3 range
XH_SCALE = 16.0           # xh -> e4m3 range
FCW_SCALE = 32.0          # fcw -> e4m3 range

_cache = {}


def _build():
    import concourse.bass as bass
    import concourse.mybir as mybir
    import concourse.tile as tile
    from concourse import bacc

    f32 = mybir.dt.float32
    bf16 = mybir.dt.bfloat16
    fp8 = mybir.dt.float8e4
    AF = mybir.ActivationFunctionType
    ALU = mybir.AluOpType
    DR = mybir.MatmulPerfMode.DoubleRow

    nc = bacc.Bacc(
        "TRN2",
        target_bir_lowering=False,
        debug=False,
        enable_asserts=False,
        num_devices=NCORES,
    )

    # DRAM parameters (per-core shapes)
    adjT_d = nc.dram_tensor("adjT", [128, KT, NS], fp8, kind="ExternalInput").ap()
    xT_d = nc.dram_tensor("xT", [PAIRS, 128, N], fp8, kind="ExternalInput").ap()
    fcw_d = nc.dram_tensor("fcw", [128, 128], fp8, kind="ExternalInput").ap()
    wzr_d = nc.dram_tensor("wzr", [128, 128], bf16, kind="ExternalInput").ap()
    mh_d = nc.dram_tensor("mh", [F_OUT, F_OUT], bf16, kind="ExternalInput").ap()
    lh_d = nc.dram_tensor("lh", [F_OUT, F_OUT], bf16, kind="ExternalInput").ap()
    bzr_d = nc.dram_tensor("bzr", [128, 1], f32, kind="ExternalInput").ap()
    bh_d = nc.dram_tensor("bh", [F_OUT, 1], f32, kind="ExternalInput").ap()
    out_d = nc.dram_tensor("out", [F_OUT, NS], f32, kind="ExternalOutput").ap()

    with tile.TileContext(nc) as tc:
        with (
            tc.tile_pool(name="const", bufs=1) as constp,
            tc.tile_pool(name="state", bufs=1) as statep,
            tc.tile_pool(name="xt", bufs=2) as xtp,
            tc.tile_pool(name="xh", bufs=2) as xhp,
            tc.tile_pool(name="gw", bufs=3) as gwp,
            tc.tile_pool(name="psx", bufs=2, space="PSUM") as psxp,
            tc.tile_pool(name="pss", bufs=2, space="PSUM") as pssp,
            tc.tile_pool(name="pszr", bufs=1, space="PSUM") as pszrp,
            tc.tile_pool(name="psh", bufs=1, space="PSUM") as pshp,
        ):
            # ---- constants ----
            fcw_sb = constp.tile([128, 128], fp8)
            nc.sync.dma_start(out=fcw_sb[:], in_=fcw_d[:])
            adjT_sb = constp.tile([128, KT, NS], fp8)
            for q, eng in enumerate((nc.sync, nc.gpsimd, nc.gpsimd, nc.gpsimd)):
                eng.dma_start(
                    out=adjT_sb[:, q * 8 : (q + 1) * 8, :],
                    in_=adjT_d[:, q * 8 : (q + 1) * 8, :],
                )
            wzr_sb = constp.tile([128, 128], bf16)
            mh_sb = constp.tile([F_OUT, F_OUT], bf16)
            lh_sb = constp.tile([F_OUT, F_OUT], bf16)
            bzr_sb = constp.tile([128, 1], f32)
            bh_sb = constp.tile([F_OUT, 1], f32)
            for dst, src in (
                (wzr_sb, wzr_d), (mh_sb, mh_d), (lh_sb, lh_d),
                (bzr_sb, bzr_d), (bh_sb, bh_d),
            ):
                nc.gpsimd.dma_start(out=dst[:], in_=src[:])

            # ---- state ----
            # Concat rhs tiles for the K=128 zr matmul: rows 0-63 carry S_t.T
            # (refreshed per pair, off-chain), rows 64-127 h.T; the combine
            # writes h' straight into the next buffer's bottom (4-rotation).
            CzS = []
            for i in range(4):
                czsi = statep.tile([128, NS], bf16, tag=f"CzS{i}")
                CzS.append(czsi)
            nc.vector.memset(CzS[0][:], 0.0)

            def emit_xh_group(xt, xh, g, eng):
                # one stacked-K matmul per 128-node tile: out[128 nodes,
                # 128 = s0 feats | s1 feats]; 8 node tiles fill 2 PSUM banks
                ps = psxp.tile([128, 1024], mybir.dt.float32)
                for j in range(8):
                    k = 8 * g + j
                    nc.tensor.matmul(
                        ps[:, j * 128 : (j + 1) * 128],
                        lhsT=xt[:, k * 128 : (k + 1) * 128],
                        rhs=fcw_sb[:],
                        start=True, stop=True,
                    )
                # xh16 = max((XH/FCW)*ps, 0) -> fp8
                dst = xh[:, 8 * g : 8 * (g + 1), :].rearrange("p a b -> p (a b)")
                if eng == "act":
                    nc.scalar.activation(dst, ps[:], AF.Relu,
                                         scale=XH_SCALE / FCW_SCALE)
                else:
                    nc.vector.tensor_scalar(dst, ps[:], XH_SCALE / FCW_SCALE,
                                            0.0, ALU.mult, ALU.max)

            def emit_gru_front(step):
                # zr matmul + combined [ZC | R] sigmoid + h*r + early Mh@S
                cur = CzS[step % 4]
                H = cur[64:128, :]

                ps_h = pshp.tile([F_OUT, NS], mybir.dt.float32, tag="ps_h")
                # early half of the h-candidate: Mh @ S (off-chain: S has
                # been sitting in cur[0:64] since the previous pair)
                nc.tensor.matmul(ps_h[:], lhsT=mh_sb[:], rhs=cur[0:64, :],
                                 start=True, stop=False)
                ps_zr = pszrp.tile([128, NS], mybir.dt.float32, tag="ps_zr")
                nc.tensor.matmul(ps_zr[:], lhsT=wzr_sb[:], rhs=cur[:],
                                 start=True, stop=True)
                # G = sigmoid(ps_zr + [-bz; br]): ZC = 1-z in rows 0-63
                # (z-columns of wzr are negated host-side), R in rows 64-127
                G = gwp.tile([128, NS], bf16, tag="G")
                nc.scalar.activation(G[:], ps_zr[:], AF.Sigmoid, bias=bzr_sb[:])
                # h*r -> base-0 tile (rhs of the late h-candidate matmul)
                ch = gwp.tile([F_OUT, NS], bf16, tag="ch")
                nc.vector.tensor_mul(ch[:], H, G[64:128, :])
                # realign ZC to partitions 64-127 for the combine (off-chain;
                # a TT with operands at different partition bases fails walrus
                # codegen, so the copy is unavoidable)
                ZC = gwp.tile([128, NS], bf16, tag="ZC")
                nc.vector.tensor_copy(ZC[64:128, :], G[0:64, :])
                G2 = gwp.tile([128, NS], bf16, tag="G2")
                nc.vector.tensor_mul(G2[64:128, :], ZC[64:128, :], H)
                P = gwp.tile([128, NS], bf16, tag="P")
                nc.vector.tensor_sub(P[64:128, :], H, G2[64:128, :])
                return ps_h, ch, ZC, P

            def emit_gru_back(step, ps_h, ch, ZC, P):
                # late h-candidate matmul + tanh + on-chain combine tail.
                # (Pinning this matmul earlier via tc.high_priority makes the
                # PE idle-wait on ch and stalls bulk work behind it: 331us
                # vs 280us -- emission-order priority is already optimal.)
                nxt = CzS[(step + 1) % 4]
                nc.tensor.matmul(ps_h[:], lhsT=lh_sb[:], rhs=ch[:],
                                 start=False, stop=True)
                HT = gwp.tile([128, NS], bf16, tag="HT")
                nc.scalar.activation(HT[64:128, :], ps_h[:], AF.Tanh,
                                     bias=bh_sb[:])
                G1 = gwp.tile([128, NS], bf16, tag="G1")
                nc.vector.tensor_mul(G1[64:128, :], ZC[64:128, :], HT[64:128, :])
                nc.vector.tensor_add(nxt[64:128, :], P[64:128, :], G1[64:128, :])

            # ---- main loop, software-pipelined: gates of pair p-1 are
            # emitted between the xh matmul groups of pair p ----
            for p in range(PAIRS):
                xt = xtp.tile([128, N], fp8)
                (nc.scalar if p % 2 == 0 else nc.sync).dma_start(
                    out=xt[:], in_=xT_d[p]
                )
                xh = xhp.tile([128, KT, 128], fp8)

                # S-pair matmul accumulator: psS[2*64 feat, 512 my-nodes],
                # 16 fp8 DoubleRow k-tile-pair MMs. Each DR MM k only needs
                # xh group k//4, so the four 4-MM bursts are interleaved
                # right after their group's relu — PE work stays dense
                # through the whole pair (no HAM re-throttle) instead of
                # bunching at the end.
                psS = pssp.tile([128, NS], mybir.dt.float32)

                def emit_s_burst(g):
                    for k in range(4 * g, 4 * g + 4):
                        nc.tensor.matmul(
                            psS[:],
                            lhsT=xh[:, 2 * k : 2 * k + 2, :],
                            rhs=adjT_sb[:, 2 * k : 2 * k + 2, :],
                            start=(k == 0), stop=(k == KT // 2 - 1),
                            perf_mode=DR,
                        )

                # order: each group's relu lands in the ACT queue BEFORE the
                # next chain op (sig/tanh), so bulk work fills chain gaps
                # instead of head-of-line blocking behind a waiting chain op
                # PE-queue placement: the chain matmuls (zr, lh) go BEFORE
                # each S-burst so they aren't stuck behind ~1us of bulk PE
                # work in the in-order FIFO when their chain deps resolve;
                # each group's relu still precedes the next chain sig/tanh
                # in the ACT queue (fills the chain gaps without blocking).
                emit_xh_group(xt, xh, 0, "act")
                if p >= 1:
                    fr0 = emit_gru_front(2 * p - 2)
                emit_xh_group(xt, xh, 1, "act")
                if p >= 1:
                    emit_gru_back(2 * p - 2, *fr0)
                emit_s_burst(0)
                # groups C and D complete BEFORE the second gru step: C's
                # relu fills the ACT gap after TANH0; D's relu runs on DVE
                # in the add0->ch1 gap. This un-gates the S2/S3 bursts and
                # the CzS copies from the chain tail (previously relu3 ran
                # on ACT after TANH1, serializing relu3->S3->copy->next zr).
                emit_xh_group(xt, xh, 2, "act")
                emit_xh_group(xt, xh, 3, "dve")
                if p >= 1:
                    fr1 = emit_gru_front(2 * p - 1)
                emit_s_burst(1)
                if p >= 1:
                    emit_gru_back(2 * p - 1, *fr1)
                emit_s_burst(2)
                emit_s_burst(3)
                # refresh concat tops for this pair's two steps (buffers
                # last read two pairs ago -> fully off the gate chain)
                s0, s1 = (2 * p) % 4, (2 * p + 1) % 4
                nc.vector.tensor_copy(CzS[s0][0:64, :], psS[0:64, :])
                nc.vector.tensor_copy(CzS[s1][0:64, :], psS[64:128, :])

            # drain: gates for the last pair
            fr = emit_gru_front(2 * PAIRS - 2)
            emit_gru_back(2 * PAIRS - 2, *fr)
            fr = emit_gru_front(2 * PAIRS - 1)
            emit_gru_back(2 * PAIRS - 1, *fr)

            Hout = statep.tile([F_OUT, NS], f32)
            nc.scalar.copy(Hout[:], CzS[(2 * PAIRS) % 4][64:128, :])
            nc.sync.dma_start(out=out_d[:], in_=Hout[:])

    nc.compile()
    return nc


def _prep_inputs(x, adj, fc_w, Wz, Wr, Wh, Lz, Lr, Lh, bz, br, bh):
    bf16 = ml_dtypes.bfloat16
    fp8 = ml_dtypes.float8_e4m3
    f32 = np.float32

    # x [T, N, F] -> [PAIRS, 2*F, N]: partition dim = (step, feat)
    xT = np.ascontiguousarray(
        x.reshape(PAIRS, 2, N, F_IN).transpose(0, 1, 3, 2).reshape(PAIRS, 128, N)
    ).astype(fp8)
    # block-diag [fcw 0; 0 fcw] so one matmul covers both steps
    fcw_stack = np.zeros((128, 128), f32)
    fcw_stack[0:64, 0:64] = fc_w * FCW_SCALE
    fcw_stack[64:128, 64:128] = fc_w * FCW_SCALE
    fcw_stack = fcw_stack.astype(fp8)

    gate_scale = 1.0 / (ADJ_SCALE * XH_SCALE)  # S arrives x65536

    def fold(W, L):
        m = W.astype(np.float64) @ L[:F_OUT].astype(np.float64)
        return (m * gate_scale).astype(np.float64)

    mz, mr, mh = fold(Wz, Lz), fold(Wr, Lr), fold(Wh, Lh)
    # zr stationary: z-columns NEGATED so sigmoid gives ZC = 1-z directly
    wzr = np.zeros((128, 128), np.float64)
    wzr[0:64, 0:64] = -mz
    wzr[64:128, 0:64] = -Lz[F_OUT:].astype(np.float64)
    wzr[0:64, 64:128] = mr
    wzr[64:128, 64:128] = Lr[F_OUT:].astype(np.float64)
    bzr = np.concatenate([-bz, br]).reshape(128, 1)
    shared = {
        "xT": xT, "fcw": fcw_stack,
        "wzr": wzr.astype(bf16),
        "mh": mh.astype(bf16),
        "lh": Lh[F_OUT:].astype(bf16),
        "bzr": bzr.astype(f32),
        "bh": bh.reshape(F_OUT, 1).astype(f32),
    }
    in_maps = []
    for c in range(NCORES):
        m = dict(shared)
        at = adj[c * NS : (c + 1) * NS, :].T * ADJ_SCALE  # [N, NS]
        m["adjT"] = np.ascontiguousarray(
            at.reshape(KT, 128, NS).transpose(1, 0, 2)
        ).astype(fp8)
        in_maps.append(m)
    return in_maps


def kernel(x, adj, fc_w, fc_b, Wz, Wr, Wh, Lz, Lr, Lh, bz, br, bh):
    x = np.asarray(x, np.float32)
    adj = np.asarray(adj, np.float32)
    args = [np.asarray(a, np.float32) for a in (fc_w, Wz, Wr, Wh, Lz, Lr, Lh, bz, br, bh)]
    fc_b = np.asarray(fc_b, np.float32)
    if np.any(fc_b != 0.0):
        # fc_b can't fold into the per-partition activation bias (it varies
        # along the free dim); the reference always passes zeros. Pure-numpy
        # fallback keeps kernel() correct for arbitrary inputs.
        return _numpy_ref(x, adj, args[0], fc_b, *args[1:])

    from concourse.bass_utils import run_bass_kernel_spmd

    if "nc" not in _cache:
        _cache["nc"] = _build()
    nc = _cache["nc"]

    in_maps = _prep_inputs(x, adj, *args)
    trace = bool(int(os.environ.get("BASS_KERNEL_TRACE", "0")))
    kwargs = {}
    if trace:
        _install_trace_shim()
        tmpdir = os.environ.get("BASS_KERNEL_TRACE_DIR")
        if tmpdir:
            os.makedirs(tmpdir, exist_ok=True)
            kwargs["tmpdir"] = tmpdir
    res = run_bass_kernel_spmd(
        nc, in_maps, core_ids=list(range(NCORES)), trace=trace, **kwargs
    )
    _cache["last_result"] = res

    out = np.empty((1, N, F_OUT), np.float32)
    for c in range(NCORES):
        out[0, c * NS : (c + 1) * NS, :] = res.results[c]["out"].T
    return out


def _install_trace_shim():
    """Register the NTFF profile hook (this image's antenv lacks axon_hooks)
    and stub out the artifact upload so profiling works offline."""
    import types

    try:
        from antenv import axon_hooks  # noqa: F401
        return
    except ImportError:
        pass
    sys.path.insert(0, "/root/.axon_site")
    from trn_agent_boot.trn_boot import _ntff_profile_via_ctypes

    hook = _ntff_profile_via_ctypes("/opt/axon/libaxon_pjrt.so")
    m = types.ModuleType("antenv.axon_hooks")
    m.get_axon_ntff_profile_hook = lambda: hook
    m.set_axon_ntff_profile_hook = lambda h: None
    sys.modules["antenv.axon_hooks"] = m
    import antenv

    antenv.axon_hooks = m
    from concourse import bass_utils as _bu

    _bu.upload_artifacts = lambda tmpdir: tmpdir


def _numpy_ref(x, adj, fc_w, fc_b, Wz, Wr, Wh, Lz, Lr, Lh, bz, br, bh):
    def sigmoid(v):
        return 1.0 / (1.0 + np.exp(-v))

    xh = np.maximum(x @ fc_w + fc_b, 0.0)
    h = np.zeros((N, F_OUT), np.float32)
    for t in range(T):
        s = adj @ xh[t]
        az, ar, ah = s @ Wz, s @ Wr, s @ Wh
        z = sigmoid(np.concatenate([az, h], -1) @ Lz + bz)
        r = sigmoid(np.concatenate([ar, h], -1) @ Lr + br)
        ht = np.tanh(np.concatenate([ah, h * r], -1) @ Lh + bh)
        h = z * h + (1.0 - z) * ht
    return h[None].astype(np.float32)
